# revision 7
# baseline (speedup 1.0000x reference)
"""Trainium2 Bass kernel for a single causal attention head (with the
faithful source bug: q = x @ W_key, W_query unused).

Full-input contract: kernel(x, W_key, W_query, W_value) -> [8, 2048, 128].
Sharding: data-parallel over batch B=8 across 8 NeuronCores (1 batch/core).

Per-core math (T=2048, C=1024, H=128):
    K = x @ W_key            (V = x @ W_value)
    S = K @ K.T * H**-0.5    (symmetric since q == k)
    out = softmax(causal(S)) @ V

Design (v2 — rebuilt around the measured bottlenecks of the previous
kernel: 136 LDWEIGHTS-bound AV matmuls, 40 small ACT chunks, 10us
GpSimd const preamble):
  - Projections contract over C in PSUM producing KT/VT [h, t]; K first,
    V of the last chunk deferred so scores start right after the last
    x byte is projected. PE warm-up matmuls during the DMA wait beat
    the p-state ramp.
  - Scores: only the upper triangle (S symmetric). Row block j =
    KT_j.T @ KT[:, j*128:] -> E layout [keys j (part), queries (free)].
    exp on ACT in 1024-wide chunks (fewer fixed overheads), diag tile
    exp'd UNMASKED and masked after the fact in fp16 SBUF (cheap 2-byte
    DVE op, off the PE->ACT critical path).
  - Denominators d_q = sum_{k<=q} E[k,q] are partition-dim sums of the
    stored blocks: row blocks are XBAR-transposed (DMA) in groups of
    two as exp proceeds, then DVE free-dim reduces + aligned adds
    accumulate d — zero PE cost, DMA/DVE only.
  - AV swapped: stationary = V tile [k, h] (constant per j), moving =
    E row blocks in 512-wide streams -> out av [h (part), q (free)]
    accumulated per 512-query chunk over j. 40 wide matmuls whose
    LDWEIGHTS hide under the streams, vs 136 small ones before.
  - av [h, q] chunks return to [q, h] via the DMA XBAR transpose
    (dma_start_transpose), normalized by recip(d) as a per-partition
    tensor_scalar_mul, and stream out per 512-row chunk.
  - V transpose for the AV stationary also via one XBAR DMA.
  - Constants (upper-tri mask) come from HBM with the weights; no
    GpSimd iota/affine_select preamble.
"""

import numpy as np

import concourse.bass as bass
import concourse.mybir as mybir
import concourse.tile as tile
from concourse import bacc, bass_utils


P = 128
T = 2048
C = 1024
H = 128
NT = T // P  # 16 seq tiles
NC = C // P  # 8 contraction tiles
NCORES = 8
SCALE = float(H) ** -0.5
F32 = mybir.dt.float32
FP16 = mybir.dt.float16
EXP = mybir.ActivationFunctionType.Exp
AXX = mybir.AxisListType.X

CHW = 512
CHN = T // CHW  # 4 chunks of the t axis during projections
NWARM = 10  # PE warm-up matmuls (p-state ramp) during the input DMA

# AV schedule: round r (issued after scores row r+1) -> list of (chunk, j)
# updates. Valid iff j <= r-1 (exp of row j finished a full round before).
# Chunk c covers queries [512c, 512c+512), needs j = 0..4c+3.
AV_SCHED = {
    4: [(0, 0), (0, 1), (0, 2), (0, 3)],
    5: [(1, 0), (1, 1), (1, 2)],
    6: [(1, 3), (1, 4), (1, 5)],
    7: [(1, 6)],
    8: [(1, 7), (2, 0), (2, 1), (2, 2)],
    9: [(2, 3), (2, 4), (2, 5), (3, 0), (3, 1)],
    10: [(2, 6), (2, 7), (2, 8), (3, 2), (3, 3)],
    11: [(2, 9), (2, 10), (3, 4), (3, 5), (3, 6)],
    12: [(2, 11), (3, 7), (3, 8), (3, 9)],
    13: [(3, 10), (3, 11), (3, 12)],
    14: [(3, 13)],
    15: [(3, 14)],
    16: [(3, 15)],
}
CLOSE_ROUND = {0: 4, 1: 8, 2: 12, 3: 16}  # round whose last (c,*) is stop


def build_module():
    nc = bacc.Bacc(
        "TRN2", target_bir_lowering=False, debug=False, num_devices=NCORES
    )
    xT_d = nc.dram_tensor("xT", [C, T], FP16, kind="ExternalInput").ap()
    # weights arranged [p, kv, c, h] on the host (one fused DMA)
    w_d = nc.dram_tensor("W", [P, 2, NC, H], FP16, kind="ExternalInput").ap()
    # upper-triangular-incl-diag mask [k, q] (1 where q >= k)
    cons_d = nc.dram_tensor("CONS", [P, P], FP16, kind="ExternalInput").ap()
    y_d = nc.dram_tensor("y", [T, H], F32, kind="ExternalOutput").ap()

    # offsets of score row-block j inside e_all (block j holds queries
    # b in [j*128, 2048) -> width (NT-j)*128)
    offs = []
    off = 0
    for j in range(NT):
        offs.append(off)
        off += (NT - j) * P
    e_width = off  # 136 * 128 = 17408

    def rw(j):  # row block width
        return (NT - j) * P

    with tile.TileContext(nc) as tc:
        with (
            tc.tile_pool(name="const", bufs=1) as const,
            tc.tile_pool(name="xt", bufs=8) as xt_pool,
            tc.tile_pool(name="kv", bufs=1) as kv,
            tc.tile_pool(name="e", bufs=1) as e_pool,
            tc.tile_pool(name="avsb", bufs=2) as avsb_pool,
            tc.tile_pool(name="ytr", bufs=2) as ytr_pool,
            tc.tile_pool(name="ysb", bufs=2) as ysb_pool,
            tc.tile_pool(name="et", bufs=2) as et_pool,
            tc.tile_pool(name="dtmp", bufs=2) as dtmp_pool,
        ):
            w_sb = const.tile([P, 2, NC, H], FP16)
            nc.sync.dma_start(w_sb[:], w_d[:])
            wk_sb = w_sb[:, 0]
            wv_sb = w_sb[:, 1]
            umask = const.tile([P, P], FP16)

            # input x chunks: triggers alternate between the two HWDGE
            # sequencers (sync / scalar), each trigger ~0.6us serialized
            xts = []
            for c in range(NC):
                xt_c = xt_pool.tile([P, T], FP16, tag="xt", name=f"xt{c}")
                eng = nc.scalar if c % 2 else nc.sync
                eng.dma_start(xt_c[:], xT_d[c * P : (c + 1) * P, :])
                xts.append(xt_c)
            nc.sync.dma_start(umask[:], cons_d[:])

            # pre-warm the ACT exp table during the input DMAs
            warm = const.tile([P, 1], F32)
            nc.vector.memset(warm[:], 0.0)
            nc.scalar.activation(warm[:], warm[:], EXP)

            kt_r = kv.tile([P, T], FP16)  # K^T [h, t]
            vt_sb = kv.tile([P, T], FP16)  # V^T [h, t]
            vaug = kv.tile([P, NT, P], FP16)  # V [t, h] per tile (XBAR)
            e_all = e_pool.tile([P, e_width], FP16)
            d_sb = kv.tile([P, NT], F32)  # softmax denominators
            recip = kv.tile([P, NT], F32)
            nc.vector.memset(d_sb[:], 0.0)

            with tc.tile_pool(name="psproj", bufs=8, space="PSUM") as psp:
                # PE warm-up: garbage matmuls to exit the low p-states
                # while the x DMA streams in.
                trash = const.tile([P, CHW], FP16)
                nc.vector.memset(trash[:], 0.0)
                warm_ps = psp.tile([P, CHW], F32, tag="ps", name="warmps")
                for i in range(NWARM):
                    nc.tensor.matmul(
                        warm_ps[:],
                        trash[:, 0:P],
                        trash[:],
                        start=True,
                        stop=True,
                    )

                kt_ps = [
                    psp.tile([P, CHW], F32, tag="ps", name=f"ktps{ch}")
                    for ch in range(CHN)
                ]
                vt_ps = [
                    psp.tile([P, CHW], F32, tag="ps", name=f"vtps{ch}")
                    for ch in range(CHN)
                ]
                # K first within each chunk; V of the last chunk deferred
                # so the kt copies (critical for scores) start earliest.
                for c in range(NC):
                    for ch in range(CHN):
                        rhs = xts[c][:, ch * CHW : (ch + 1) * CHW]
                        nc.tensor.matmul(
                            kt_ps[ch][:],
                            wk_sb[:, c, :],
                            rhs,
                            start=(c == 0),
                            stop=(c == NC - 1),
                        )
                    if c < NC - 1:
                        for ch in range(CHN):
                            rhs = xts[c][:, ch * CHW : (ch + 1) * CHW]
                            nc.tensor.matmul(
                                vt_ps[ch][:],
                                wv_sb[:, c, :],
                                rhs,
                                start=(c == 0),
                                stop=False,
                            )
                # kt -> SBUF fp16 (DVE), in score-consumption order
                for ch in range(CHN):
                    sl = slice(ch * CHW, (ch + 1) * CHW)
                    nc.vector.tensor_copy(kt_r[:, sl], kt_ps[ch][:])
                # deferred V of the last chunk, then vt copies + XBAR
                for ch in range(CHN):
                    rhs = xts[NC - 1][:, ch * CHW : (ch + 1) * CHW]
                    nc.tensor.matmul(
                        vt_ps[ch][:],
                        wv_sb[:, NC - 1, :],
                        rhs,
                        start=False,
                        stop=True,
                    )
                for ch in range(CHN):
                    sl = slice(ch * CHW, (ch + 1) * CHW)
                    nc.vector.tensor_copy(vt_sb[:, sl], vt_ps[ch][:])
                # V^T [h, t] -> per-tile V [t, h] via the DMA crossbar
                nc.sync.dma_start_transpose(vaug[:], vt_sb[:])

            with (
                tc.tile_pool(name="pssc", bufs=3, space="PSUM") as pssc,
                tc.tile_pool(name="psav", bufs=2, space="PSUM") as psav,
            ):
                av_ps = {}

                def scores_row(j):
                    """Issue score matmuls + exp for row block j in
                    1024-wide PSUM tiles; mask diag + reduce d after."""
                    b0 = j * P
                    width = rw(j)
                    pos = 0
                    while pos < width:
                        wt = min(1024, width - pos)
                        s_ps = pssc.tile(
                            [P, 1024], F32, tag="ps", name=f"sps{j}_{pos}"
                        )
                        p2 = 0
                        while p2 < wt:
                            w2 = min(CHW, wt - p2)
                            nc.tensor.matmul(
                                s_ps[:, p2 : p2 + w2],
                                kt_r[:, b0 : b0 + P],
                                kt_r[:, b0 + pos + p2 : b0 + pos + p2 + w2],
                                start=True,
                                stop=True,
                            )
                            p2 += w2
                        nc.scalar.activation(
                            e_all[:, offs[j] + pos : offs[j] + pos + wt],
                            s_ps[:, :wt],
                            EXP,
                            scale=SCALE,
                        )
                        pos += wt

                def mask_row(j):
                    # causal mask on the diag tile (post-exp, fp16)
                    nc.vector.tensor_mul(
                        e_all[:, offs[j] : offs[j] + P],
                        e_all[:, offs[j] : offs[j] + P],
                        umask[:],
                    )

                # d groups: blocks (2g, 2g+1) XBAR-transposed together
                # once both rows' diag masks applied; DVE then reduces
                # each transposed block over k and adds into d aligned
                # by query tile.
                et_tiles = {}

                def d_xbar(g):
                    j0, j1 = 2 * g, 2 * g + 1
                    ntg = (NT - j0) + (NT - j1)
                    et = et_pool.tile(
                        [P, 31, P], FP16, tag="et", name=f"et{g}"
                    )
                    nc.sync.dma_start_transpose(
                        et[:, 0:ntg, :],
                        e_all[:, offs[j0] : offs[j0] + ntg * P],
                    )
                    et_tiles[g] = et

                def d_reduce(g):
                    et = et_tiles[g]
                    base = 0
                    for j in (2 * g, 2 * g + 1):
                        ntj = NT - j
                        dt = dtmp_pool.tile(
                            [P, NT, 1], F32, tag="dt", name=f"dt{j}"
                        )
                        nc.vector.reduce_sum(
                            dt[:, 0:ntj, :],
                            et[:, base : base + ntj, :],
                            axis=AXX,
                        )
                        nc.vector.tensor_add(
                            d_sb[:, j:NT],
                            d_sb[:, j:NT],
                            dt[:, 0:ntj, 0],
                        )
                        base += ntj

                def av_update(c, j):
                    q0 = c * 512  # chunk origin in queries
                    boff = q0 - j * P  # offset inside row block j
                    if boff >= 0:
                        wt = 512
                        osl = slice(0, 512)
                    else:
                        wt = 512 + boff
                        osl = slice(512 - wt, 512)
                        boff = 0
                    nc.tensor.matmul(
                        av_ps[c][:, osl],
                        vaug[:, j, :],
                        e_all[:, offs[j] + boff : offs[j] + boff + wt],
                        start=(j == 0),
                        stop=(j == min(4 * c + 3, NT - 1)),
                    )

                ytr_tiles = {}

                def drain_chunk(c):
                    """After chunk c's accumulation closed: PSUM->SBUF
                    fp16 (frees the bank) + XBAR back-transpose."""
                    av_sb = avsb_pool.tile(
                        [P, 512], FP16, tag="avsb", name=f"avsb{c}"
                    )
                    nc.vector.tensor_copy(av_sb[:], av_ps[c][:])
                    ytr = ytr_pool.tile(
                        [P, 4, P], FP16, tag="ytr", name=f"ytr{c}"
                    )
                    nc.sync.dma_start_transpose(ytr[:], av_sb[:])
                    ytr_tiles[c] = ytr

                def out_chunk(c):
                    """recip (d complete for this chunk), normalize,
                    stream out."""
                    nc.vector.reciprocal(
                        recip[:, 4 * c : 4 * c + 4], d_sb[:, 4 * c : 4 * c + 4]
                    )
                    ytr = ytr_tiles[c]
                    y_sb = ysb_pool.tile(
                        [P, 4, P], F32, tag="ysb", name=f"ysb{c}"
                    )
                    for i in range(4):
                        nc.vector.tensor_scalar_mul(
                            y_sb[:, i, :],
                            ytr[:, i, :],
                            recip[:, 4 * c + i : 4 * c + i + 1],
                        )
                    y_view = y_d[512 * c : 512 * (c + 1), :].rearrange(
                        "(i p) h -> p i h", p=P
                    )
                    nc.sync.dma_start(y_view, y_sb[:])

                # software pipeline: round r issues scores row r+1 first,
                # then AV updates for rows exp'd at least a round ago.
                # d-group g: XBAR fires at round 2g+1 (its masks done),
                # reduces land at round 2g+2; chunk c's d is complete
                # after group 2c+1 -> out_chunk(c) at round 4c+4/4c+5.
                scores_row(0)
                for r in range(NT + 2):
                    if r + 1 < NT:
                        scores_row(r + 1)
                    for c, j in AV_SCHED.get(r, ()):
                        if j == 0:
                            av_ps[c] = psav.tile(
                                [P, 512], F32, tag="ps", name=f"avps{c}"
                            )
                        av_update(c, j)
                    if r < NT:
                        mask_row(r)
                    if r % 2 == 1 and r < NT:  # rounds 1,3,..,15
                        d_xbar((r - 1) // 2)
                    if r % 2 == 0 and 2 <= r <= NT:  # rounds 2,4,..,16
                        d_reduce((r - 2) // 2)
                    for c, rr in CLOSE_ROUND.items():
                        if rr == r:
                            drain_chunk(c)
                        if rr == r - 1:
                            out_chunk(c)

    nc.compile()
    return nc


_NC_CACHE = None


def _get_module():
    global _NC_CACHE
    if _NC_CACHE is None:
        _NC_CACHE = build_module()
    return _NC_CACHE


def run(in_maps, trace=False, **kw):
    nc = _get_module()
    return bass_utils.run_bass_kernel_spmd(
        nc, in_maps, core_ids=list(range(NCORES)), trace=trace, **kw
    )


def make_in_maps(x, W_key, W_value):
    x = np.asarray(x, dtype=np.float32).astype(np.float16)
    xT = np.ascontiguousarray(x.transpose(0, 2, 1))
    wk = np.asarray(W_key, np.float32).astype(np.float16)
    wk = wk.reshape(NC, P, H).transpose(1, 0, 2)
    wv = np.asarray(W_value, np.float32).astype(np.float16)
    wv = wv.reshape(NC, P, H).transpose(1, 0, 2)
    w = np.ascontiguousarray(np.stack([wk, wv], axis=1))  # [P, 2, NC, H]
    # [k, q] keep where q >= k
    umask = np.triu(np.ones((P, P), dtype=np.float16))
    umask = np.ascontiguousarray(umask)
    return [{"xT": xT[b], "W": w, "CONS": umask} for b in range(NCORES)]


def kernel(x, W_key, W_query, W_value):
    # W_query intentionally unused: the reference applies W_key for q too.
    del W_query
    res = run(make_in_maps(x, W_key, W_value), trace=False)
    return np.stack([res.results[b]["y"] for b in range(NCORES)], axis=0)


# revision 23
# speedup vs baseline: 1.3815x; 1.3815x over previous
"""Trainium2 Bass kernel for a single causal attention head (with the
faithful source bug: q = x @ W_key, W_query unused).

Full-input contract: kernel(x, W_key, W_query, W_value) -> [8, 2048, 128].
Sharding: data-parallel over batch B=8 across 8 NeuronCores (1 batch/core).

Per-core math (T=2048, C=1024, H=128):
    K = x @ W_key            (V = x @ W_value)
    S = K @ K.T * H**-0.5    (symmetric since q == k)
    out = softmax(causal(S)) @ V

v3 design (profile-driven rebuild of the original kernel):
  - PE warm-up matmuls during the ~10us input-DMA window beat the
    p-state ramp so real matmuls start at full clock.
  - Projections contract over C in PSUM producing KT/VT [h, t]; K of
    the last chunk prioritized and V of the last chunk deferred so the
    kt PSUM->SBUF casts (critical path to scores) start immediately
    after the last x byte. kt casts on DVE, vt casts split DVE/ACT.
  - V^T -> per-tile V [t, h] via ONE whole-tensor XBAR DMA transpose
    (validated on HW), then a strided copy into the 129-wide vaug
    whose last column is ones — the ones ride the AV matmul so the
    softmax denominator accumulates for free (the v1 trick).
  - Scores: upper triangle only (S symmetric). exp on ACT in 1024-wide
    PSUM tiles (24 ACTIVATEs instead of 40 — ACT fixed costs are the
    mid-phase critical path), diag tile exp'd unmasked and masked
    after in fp16 (cheap 2-byte DVE op off the PE->ACT chain).
  - AV per (j, i): stationary E_ji [k, q in tile i], moving
    vaug_j [k, 129]; av column i accumulates in its own PSUM bank.
    Catch-up schedule front-loads early rounds (ACT-bound) so the PE
    has slack when rounds get PE-bound near the causal triangle base.
  - Outputs batch 4 seq tiles per DMA (4 output DMAs instead of 16 —
    DIRECT2D triggers cost ~0.7us of engine time each).
"""

import numpy as np

import concourse.bass as bass
import concourse.mybir as mybir
import concourse.tile as tile
from concourse import bacc, bass_utils


P = 128
T = 2048
C = 1024
H = 128
NT = T // P  # 16 seq tiles
NC = C // P  # 8 contraction tiles
NCORES = 8
NAV = P + 1  # v | ones
SCALE = float(H) ** -0.5
F32 = mybir.dt.float32
FP16 = mybir.dt.float16
EXP = mybir.ActivationFunctionType.Exp

CHW = 512
CHN = T // CHW  # 4 chunks of the t axis during projections
NWARM = 6  # PE warm-up matmuls (p-state ramp) during the input DMA


def build_av_schedule():
    """AV update (j, i) -> round. Column i's bank is live from round
    max(1, i-3) to i (stop at j == i); updates front-loaded into the
    earlier (ACT-bound) rounds. Safe when j <= round-1; j == round
    only stalls briefly on the concurrent exp."""
    sched = {}
    for i in range(NT):
        # columns 0/1 in round 1; later columns from round 2 so the
        # 4-slot PSUM rotation always sees the previous tenant's drain
        # (emitted at the top of round i-3) before the new first write
        rounds = list(range(max(2, i - 3), i + 1)) if i >= 2 else [1]
        js = list(range(i + 1))
        k = len(rounds)
        base, rem = divmod(len(js), k)
        sizes = [base + (1 if x < rem else 0) for x in range(k)]
        pos = 0
        for r, sz in zip(rounds, sizes):
            for j in js[pos : pos + sz]:
                sched.setdefault(r, []).append((j, i))
            pos += sz
    return sched


AV_SCHED = build_av_schedule()


def build_module():
    nc = bacc.Bacc(
        "TRN2", target_bir_lowering=False, debug=False, num_devices=NCORES
    )
    xT_d = nc.dram_tensor("xT", [C, T], FP16, kind="ExternalInput").ap()
    # weights arranged [p, kv, c, h] on the host (one fused DMA)
    w_d = nc.dram_tensor("W", [P, 2, NC, H], FP16, kind="ExternalInput").ap()
    # [:, 0] upper-tri-incl-diag mask [k, q]; [:, 1] identity
    cons_d = nc.dram_tensor("CONS", [P, 2, P], FP16, kind="ExternalInput").ap()
    y_d = nc.dram_tensor("y", [T, H], F32, kind="ExternalOutput").ap()

    # offsets of score row-block j inside e_all (block j holds queries
    # b in [j*128, 2048) -> width (NT-j)*128)
    offs = []
    off = 0
    for j in range(NT):
        offs.append(off)
        off += (NT - j) * P
    e_width = off  # 136 * 128 = 17408

    def rw(j):  # row block width
        return (NT - j) * P

    with tile.TileContext(nc) as tc:
        with (
            tc.tile_pool(name="const", bufs=1) as const,
            tc.tile_pool(name="xt", bufs=8) as xt_pool,
            tc.tile_pool(name="kv", bufs=1) as kv,
            tc.tile_pool(name="e", bufs=1) as e_pool,
            tc.tile_pool(name="ysb", bufs=2) as ysb_pool,
            tc.tile_pool(name="rcp", bufs=4) as rcp_pool,
        ):
            w_sb = const.tile([P, 2, NC, H], FP16)
            nc.sync.dma_start(w_sb[:], w_d[:])
            wk_sb = w_sb[:, 0]
            wv_sb = w_sb[:, 1]
            cons = const.tile([P, 2, P], FP16)

            # input x chunks: triggers alternate between the two HWDGE
            # sequencers (sync / scalar), each trigger ~0.7us serialized
            xts = []
            for c in range(NC):
                xt_c = xt_pool.tile([P, T], FP16, tag="xt", name=f"xt{c}")
                eng = nc.scalar if c % 2 else nc.sync
                eng.dma_start(xt_c[:], xT_d[c * P : (c + 1) * P, :])
                xts.append(xt_c)
            nc.sync.dma_start(cons[:], cons_d[:])
            umask = cons[:, 0]

            # pre-warm the ACT exp table during the input DMAs
            warm = const.tile([P, 1], F32)
            nc.vector.memset(warm[:], 0.0)
            nc.scalar.activation(warm[:], warm[:], EXP)

            kt_r = kv.tile([P, T], FP16)  # K^T [h, t]
            vt_sb = kv.tile([P, T], FP16)  # V^T [h, t]
            vtmp = kv.tile([P, NT, P], FP16)  # XBAR dst: V [t, h] tiles
            vaug = kv.tile([P, NT, NAV], FP16)  # [v | ones]
            nc.vector.memset(vaug[:], 1.0)  # ones cols; v overwritten
            e_all = e_pool.tile([P, e_width], FP16)

            with tc.tile_pool(name="psproj", bufs=8, space="PSUM") as psp:
                # PE warm-up: garbage matmuls to exit the low p-states
                # while the x DMA streams in.
                trash = const.tile([P, CHW], FP16)
                nc.vector.memset(trash[:], 0.0)
                warm_ps = psp.tile([P, CHW], F32, tag="ps", name="warmps")
                for _ in range(NWARM):
                    nc.tensor.matmul(
                        warm_ps[:],
                        trash[:, 0:P],
                        trash[:],
                        start=True,
                        stop=True,
                    )

                kt_ps = [
                    psp.tile([P, CHW], F32, tag="ps", name=f"ktps{ch}")
                    for ch in range(CHN)
                ]
                vt_ps = [
                    psp.tile([P, CHW], F32, tag="ps", name=f"vtps{ch}")
                    for ch in range(CHN)
                ]
                # K first within each chunk; V of the last chunk deferred
                # so the kt casts (critical for scores) start earliest.
                for c in range(NC):
                    for ch in range(CHN):
                        rhs = xts[c][:, ch * CHW : (ch + 1) * CHW]
                        nc.tensor.matmul(
                            kt_ps[ch][:],
                            wk_sb[:, c, :],
                            rhs,
                            start=(c == 0),
                            stop=(c == NC - 1),
                        )
                    if c < NC - 1:
                        for ch in range(CHN):
                            rhs = xts[c][:, ch * CHW : (ch + 1) * CHW]
                            nc.tensor.matmul(
                                vt_ps[ch][:],
                                wv_sb[:, c, :],
                                rhs,
                                start=(c == 0),
                                stop=False,
                            )
                # kt -> SBUF fp16 (DVE), in score-consumption order
                for ch in range(CHN):
                    sl = slice(ch * CHW, (ch + 1) * CHW)
                    nc.vector.tensor_copy(kt_r[:, sl], kt_ps[ch][:])
                # deferred V of the last chunk, then vt casts + XBAR
                for ch in range(CHN):
                    rhs = xts[NC - 1][:, ch * CHW : (ch + 1) * CHW]
                    nc.tensor.matmul(
                        vt_ps[ch][:],
                        wv_sb[:, NC - 1, :],
                        rhs,
                        start=False,
                        stop=True,
                    )
                # vt casts split: ACT takes 2 (idle pre-exp), DVE 2.
                # V^T [h, t] -> per-tile V [t, h] via the DMA crossbar,
                # in two halves so the first fires as soon as the ACT
                # casts land (the XBAR->vaug chain gates the first AV).
                nc.scalar.copy(vt_sb[:, 0:CHW], vt_ps[0][:])
                nc.scalar.copy(vt_sb[:, CHW : 2 * CHW], vt_ps[1][:])
                nc.sync.dma_start_transpose(
                    vtmp[:, 0 : NT // 2, :], vt_sb[:, 0 : T // 2]
                )
                nc.vector.tensor_copy(vt_sb[:, 2 * CHW : 3 * CHW], vt_ps[2][:])
                nc.vector.tensor_copy(vt_sb[:, 3 * CHW : 4 * CHW], vt_ps[3][:])
                nc.sync.dma_start_transpose(
                    vtmp[:, NT // 2 : NT, :], vt_sb[:, T // 2 : T]
                )

            with (
                tc.tile_pool(name="pssc", bufs=2, space="PSUM") as pssc,
                tc.tile_pool(name="psav", bufs=4, space="PSUM") as psav,
            ):
                av_ps = {}
                y_tiles = {}

                def scores_row(j):
                    """Score matmuls + exp for row block j in 1024-wide
                    PSUM tiles (2 matmuls, 1 ACTIVATE per tile)."""
                    b0 = j * P
                    width = rw(j)
                    pos = 0
                    while pos < width:
                        wt = min(1024, width - pos)
                        s_ps = pssc.tile(
                            [P, 1024], F32, tag="ps", name=f"sps{j}_{pos}"
                        )
                        p2 = 0
                        while p2 < wt:
                            w2 = min(CHW, wt - p2)
                            nc.tensor.matmul(
                                s_ps[:, p2 : p2 + w2],
                                kt_r[:, b0 : b0 + P],
                                kt_r[:, b0 + pos + p2 : b0 + pos + p2 + w2],
                                start=True,
                                stop=True,
                            )
                            p2 += w2
                        nc.scalar.activation(
                            e_all[:, offs[j] + pos : offs[j] + pos + wt],
                            s_ps[:, :wt],
                            EXP,
                            scale=SCALE,
                        )
                        pos += wt

                def mask_row(j):
                    # causal mask on the diag tile (post-exp, fp16)
                    nc.vector.tensor_mul(
                        e_all[:, offs[j] : offs[j] + P],
                        e_all[:, offs[j] : offs[j] + P],
                        umask[:],
                    )

                def av_update(j, i):
                    if j == 0:
                        av_ps[i] = psav.tile(
                            [P, CHW], F32, tag="ps", name=f"avps{i}"
                        )
                    eji = e_all[
                        :, offs[j] + (i - j) * P : offs[j] + (i - j + 1) * P
                    ]
                    nc.tensor.matmul(
                        av_ps[i][:, 0:NAV],
                        eji,
                        vaug[:, j, :],
                        start=(j == 0),
                        stop=(j == i),
                    )

                def drain_col(i):
                    """Column i closed last round: normalize by the
                    ones-column denominator into the group y tile."""
                    g = i // 4
                    if i % 4 == 0:
                        y_tiles[g] = ysb_pool.tile(
                            [P, 4, P], F32, tag="ysb", name=f"ysb{g}"
                        )
                    av = av_ps[i]
                    recip = rcp_pool.tile(
                        [P, 1], F32, tag="recip", name=f"rcp{i}"
                    )
                    nc.vector.reciprocal(recip[:], av[:, P : P + 1])
                    nc.vector.tensor_scalar_mul(
                        y_tiles[g][:, i % 4, :], av[:, 0:P], recip[:]
                    )
                    if i % 4 == 3:
                        y_view = y_d[512 * g : 512 * (g + 1), :].rearrange(
                            "(i p) h -> p i h", p=P
                        )
                        nc.sync.dma_start(y_view, y_tiles[g][:])

                # vaug = [vtmp | ones]: per-tile contiguous copies once
                # the XBAR lands (strided full-tensor copy miscompiles),
                # emitted incrementally a few tiles ahead of AV use.
                vaug_done = [0]

                def vaug_copy_upto(n):
                    while vaug_done[0] < min(n, NT):
                        j = vaug_done[0]
                        nc.vector.tensor_copy(
                            vaug[:, j, 0:P], vtmp[:, j, :]
                        )
                        vaug_done[0] += 1

                # software pipeline: round r issues scores row r+1,
                # then the AV catch-up batch for live columns.
                scores_row(0)
                for r in range(NT + 2):
                    # drain first: column i closes at round max(1, i),
                    # and its PSUM slot is re-allocated 4 columns later —
                    # the drain reads must be emitted before the new
                    # column's first write for Tile to see the WAR.
                    if r == 2:
                        drain_col(0)
                    if 2 <= r <= NT:
                        drain_col(r - 1)
                    # mask before any AV update may read row r's diag
                    if r < NT:
                        mask_row(r)
                    vaug_copy_upto(4 + 2 * r)
                    batch = AV_SCHED.get(r, ())
                    if r > 1:
                        for j, i in batch[:2]:
                            av_update(j, i)
                        batch = batch[2:]
                    if r + 1 < NT:
                        scores_row(r + 1)
                    for j, i in batch:
                        av_update(j, i)

    nc.compile()
    return nc


_NC_CACHE = None


def _get_module():
    global _NC_CACHE
    if _NC_CACHE is None:
        _NC_CACHE = build_module()
    return _NC_CACHE


def run(in_maps, trace=False, **kw):
    nc = _get_module()
    return bass_utils.run_bass_kernel_spmd(
        nc, in_maps, core_ids=list(range(NCORES)), trace=trace, **kw
    )


def make_in_maps(x, W_key, W_value):
    x = np.asarray(x, dtype=np.float32).astype(np.float16)
    xT = np.ascontiguousarray(x.transpose(0, 2, 1))
    wk = np.asarray(W_key, np.float32).astype(np.float16)
    wk = wk.reshape(NC, P, H).transpose(1, 0, 2)
    wv = np.asarray(W_value, np.float32).astype(np.float16)
    wv = wv.reshape(NC, P, H).transpose(1, 0, 2)
    w = np.ascontiguousarray(np.stack([wk, wv], axis=1))  # [P, 2, NC, H]
    umask = np.triu(np.ones((P, P), dtype=np.float16))  # keep q >= k
    ident = np.eye(P, dtype=np.float16)
    cons = np.ascontiguousarray(np.stack([umask, ident], axis=1))
    return [{"xT": xT[b], "W": w, "CONS": cons} for b in range(NCORES)]


def kernel(x, W_key, W_query, W_value):
    # W_query intentionally unused: the reference applies W_key for q too.
    del W_query
    res = run(make_in_maps(x, W_key, W_value), trace=False)
    return np.stack([res.results[b]["y"] for b in range(NCORES)], axis=0)
